# revision 19
# baseline (speedup 1.0000x reference)
"""Hamming-distance kernel for Trainium2 (8 NeuronCores, SPMD).

out[n, m] = mean_d(x[n, d] != y[m, d]),  x: (8192, 256), y: (8192, 256),
values are small integers 0..7 stored as float32.

Formulation: categorical equality as a +-1 Hadamard-code GEMM.  Each value
c in {0..7} maps to the 7 non-constant entries of row c of the 8x8
Hadamard matrix: had_j(c) = (-1)^popcount(c & k_j), k_j in {1..7}.  Rows
satisfy <h(a), h(b)> = 8*[a==b] - 1, so with dot[n,m] over K = 7*256 =
1792 features:  eq = (dot + 256)/8  and  out = 1 - eq/256 = 0.875 -
dot/2048.  All code values are +-1 (exact in fp8e4), PSUM accumulates in
fp32 (|dot| <= 1792 << 2^24), and 0.875 - dot*2^-11 is exact binary
arithmetic, so the result is bit-exact.  K = 7 per dim is the provable
minimum embedding for exact categorical equality (vs 8 for one-hot).

Sharding: x rows split across 8 cores (1024 rows each), y replicated.
Each core computes a (1024, 8192) slice of the output.

Device pipeline per core:
  1. DMA x^T shard (256, 1024) and y^T (256, 8192) f32 into fresh SBUF
     slots (host supplies the transposed layout; the feature dim must sit
     on SBUF partitions for the contraction).  Fresh slots => each DMA
     needs at most one sem wait (the DMA ISA has a single wait slot).
  2. Encode +-1 codes k-major: per (m-chunk, d-half): two DVE mods
     (v mod 2, v mod 4), three ACT Sign ops (bit signs s1, s2, s4), four
     DVE products (s3, s5, s6, s7).  y is encoded per m-group into a ring
     of fp8 chunk tiles so encode overlaps the previous group's matmuls.
  3. fp8 DoubleRow GEMM: psum[128, 512] accumulated over 7 k-pairs.
  4. ACT-engine PSUM eviction fused with the affine map 0.875 - dot/2048.
"""

import numpy as np

import concourse.bacc as bacc
import concourse.bass as bass
import concourse.mybir as mybir
import concourse.tile as tile
from concourse.bass_utils import run_bass_kernel_spmd

# Problem dims (hardcoded per contract).
N, M, D, C = 8192, 8192, 256, 8
N_CORES = 8
N_SH = N // N_CORES  # 1024 x-rows per core

P = 128
D_HALVES = D // P  # 2
N_CODES = 7  # +-1 Hadamard code length per dim
KSUB = N_CODES * D_HALVES  # 14 k-subtiles of 128 features -> K = 1792
K_PAIRS = KSUB // 2  # 7 DoubleRow pairs (256 contracted per matmul)
M_CHUNK = 512  # output free-dim tile (one PSUM bank of f32)
M_CHUNKS = M // M_CHUNK  # 16
N_TILES = N_SH // P  # 8
M_GROUP = 4  # m-chunks per psum group (4 banks busy, 8 total)
M_GROUPS = M_CHUNKS // M_GROUP  # 4
MG_COLS = M_GROUP * M_CHUNK  # 2048 m-columns per group

FP8 = mybir.dt.float8e4
F32 = mybir.dt.float32
I32 = mybir.dt.int32
ALU = mybir.AluOpType
ACTF = mybir.ActivationFunctionType


def _encode_half(nc, tmp_pool, dst, h, src, w, biases):
    """Write the 7 +-1 code tiles for d-half h of raw tile src[:, h, :w]
    into dst[:, 2*j + h, :w], j = 0..6.

    Code order j: masks [1, 2, 4, 3, 5, 6, 7]; s_k(v) = (-1)^popcount(v&k).
    s1/s2/s4 from bit signs (ACT Sign), rest are DVE products.
    """
    v = src[:, h, :w]
    b05, b15, b35 = biases

    def slot(j):
        return dst[:, 2 * j + h, :w]

    vi = tmp_pool.tile([P, w], I32, name="enc_vi")
    nc.vector.tensor_copy(vi[:], v)
    t0 = tmp_pool.tile([P, w], I32, name="enc_t0")
    nc.vector.tensor_scalar(
        out=t0[:], in0=vi[:], scalar1=1, scalar2=None, op0=ALU.bitwise_and
    )
    u = tmp_pool.tile([P, w], I32, name="enc_u")
    nc.vector.tensor_scalar(
        out=u[:], in0=vi[:], scalar1=2, scalar2=None, op0=ALU.bitwise_and
    )
    s1, s2, s4 = slot(0), slot(1), slot(2)
    # sign(-t + b): +1 when bit clear, -1 when set
    nc.scalar.activation(s1, t0[:], ACTF.Sign, bias=b05[:], scale=-1.0)
    nc.scalar.activation(s2, u[:], ACTF.Sign, bias=b15[:], scale=-1.0)
    nc.scalar.activation(s4, v, ACTF.Sign, bias=b35[:], scale=-1.0)
    s3, s5, s6, s7 = slot(3), slot(4), slot(5), slot(6)
    nc.vector.tensor_tensor(s3, s1, s2, ALU.mult)
    nc.vector.tensor_tensor(s5, s1, s4, ALU.mult)
    nc.vector.tensor_tensor(s6, s2, s4, ALU.mult)
    nc.vector.tensor_tensor(s7, s3, s4, ALU.mult)


def _build_bass(repeats: int = 1):
    # Bacc (not raw Bass): its compile() legalizes multi-semaphore waits
    # into EventSemaphore instructions (HW allows 1 wait per instruction).
    nc = bacc.Bacc(
        "TRN2", target_bir_lowering=False, debug=False, num_devices=N_CORES
    )

    xt_d = nc.dram_tensor("xt", [D, N_SH], F32, kind="ExternalInput")
    yt_d = nc.dram_tensor("yt", [D, M], F32, kind="ExternalInput")
    # Blocked output layout: block (n, mc) is one contiguous 128x512 f32
    # region, so store DMAs are interval-disjoint (no false WAW chains) and
    # fully contiguous.  The host de-blocks with a transpose+reshape.
    out_d = nc.dram_tensor(
        "out", [N_TILES, M_CHUNKS, P, M_CHUNK], F32, kind="ExternalOutput"
    )

    xt_r = xt_d.rearrange("(h p) n -> p h n", p=P)
    yt_r = yt_d.rearrange("(h p) m -> p h m", p=P)

    with tile.TileContext(nc) as tc:
        with (
            tc.tile_pool(name="xe", bufs=1) as xe_pool,
            tc.tile_pool(name="ye", bufs=2 * M_GROUP) as ye_pool,
            tc.tile_pool(name="xraw", bufs=1) as xraw_pool,
            tc.tile_pool(name="yraw", bufs=M_GROUPS) as yraw_pool,
            tc.tile_pool(name="tmp", bufs=4) as tmp_pool,
            tc.tile_pool(name="out", bufs=8) as out_pool,
            tc.tile_pool(name="psum", bufs=8, space="PSUM") as psum_pool,
        ):
            # ---- bias constants for ACT Sign ----
            biases = []
            for val in (0.5, 1.5, 3.5):
                b = tmp_pool.tile([P, 1], F32, name=f"bias_{val}", bufs=1)
                nc.vector.memset(b[:], val)
                biases.append(b)

            # ---- raw loads: all into fresh slots ----
            xt_sb = xraw_pool.tile([P, D_HALVES, N_SH], F32)
            nc.sync.dma_start(xt_sb[:], xt_r)
            yraw_tiles = []
            for mg in range(M_GROUPS):
                yt_sb = yraw_pool.tile([P, D_HALVES, MG_COLS], F32, name="yt_sb")
                nc.sync.dma_start(
                    yt_sb[:], yt_r[:, :, mg * MG_COLS : (mg + 1) * MG_COLS]
                )
                yraw_tiles.append(yt_sb)

            # ---- x codes ----
            xe = xe_pool.tile([P, KSUB, N_SH], FP8)
            for h in range(D_HALVES):
                _encode_half(nc, tmp_pool, xe, h, xt_sb, N_SH, biases)

            def _one_pass():
                for mg in range(M_GROUPS):
                    # y codes for this m-group (ring; encode of group g+1
                    # overlaps matmuls of group g)
                    ye_tiles = []
                    for j in range(M_GROUP):
                        ye_mc = ye_pool.tile([P, KSUB, M_CHUNK], FP8, name="ye_mc")
                        for h in range(D_HALVES):
                            _encode_half(
                                nc,
                                tmp_pool,
                                ye_mc,
                                h,
                                yraw_tiles[mg][
                                    :, :, j * M_CHUNK : (j + 1) * M_CHUNK
                                ],
                                M_CHUNK,
                                biases,
                            )
                        ye_tiles.append(ye_mc)

                    for n in range(N_TILES):
                        psum_tiles = [
                            psum_pool.tile([P, M_CHUNK], F32, name="psum")
                            for _ in range(M_GROUP)
                        ]
                        for kp in range(K_PAIRS):
                            lhsT = xe[:, 2 * kp : 2 * kp + 2, n * P : (n + 1) * P]
                            for j in range(M_GROUP):
                                nc.tensor.matmul(
                                    psum_tiles[j][:],
                                    lhsT,
                                    ye_tiles[j][:, 2 * kp : 2 * kp + 2, :],
                                    start=(kp == 0),
                                    stop=(kp == K_PAIRS - 1),
                                    perf_mode=mybir.MatmulPerfMode.DoubleRow,
                                )
                        for j in range(M_GROUP):
                            ot = out_pool.tile([P, M_CHUNK], F32, name="ot")
                            # out = 0.875 - dot/2048  (exact)
                            nc.scalar.activation(
                                ot[:],
                                psum_tiles[j][:],
                                ACTF.Copy,
                                bias=0.875,
                                scale=-1.0 / 2048.0,
                            )
                            mc = mg * M_GROUP + j
                            nc.sync.dma_start(out_d[n, mc], ot[:])

            if repeats == 1:
                _one_pass()
            else:
                # device-side repeat loop, used only for wall-clock timing
                with tc.For_i(0, repeats, 1):
                    _one_pass()
    nc.compile()
    return nc


_NC_CACHE = {}


def _get_nc(repeats: int = 1):
    if repeats not in _NC_CACHE:
        _NC_CACHE[repeats] = _build_bass(repeats)
    return _NC_CACHE[repeats]


def _make_in_maps(x: np.ndarray, y: np.ndarray):
    yt = np.ascontiguousarray(y.T)  # (256, 8192)
    in_maps = []
    for i in range(N_CORES):
        xt_i = np.ascontiguousarray(x[i * N_SH : (i + 1) * N_SH].T)  # (256, 1024)
        in_maps.append({"xt": xt_i, "yt": yt})
    return in_maps


def _deblock(blocked: np.ndarray) -> np.ndarray:
    # (N_TILES, M_CHUNKS, P, M_CHUNK) -> (N_SH, M)
    return np.ascontiguousarray(
        blocked.transpose(0, 2, 1, 3).reshape(N_SH, M)
    )


def kernel(x: np.ndarray, y: np.ndarray) -> np.ndarray:
    x = np.asarray(x, dtype=np.float32)
    y = np.asarray(y, dtype=np.float32)
    assert x.shape == (N, D) and y.shape == (M, D)

    nc = _get_nc(1)
    in_maps = _make_in_maps(x, y)
    res = run_bass_kernel_spmd(nc, in_maps, core_ids=list(range(N_CORES)))
    return np.concatenate(
        [_deblock(r["out"]) for r in res.results], axis=0
    )
